# revision 13
# baseline (speedup 1.0000x reference)
import os
import sys

import numpy as np

sys.path.insert(0, "/opt/trn_rl_repo")

import ml_dtypes  # noqa: E402

import concourse.bass as bass  # noqa: E402
import concourse.tile as tile  # noqa: E402
from concourse import bacc, mybir  # noqa: E402
from concourse.bass_utils import run_bass_kernel_spmd  # noqa: E402

BF16 = ml_dtypes.bfloat16
N, C, G = 500000, 128, 4096
NCORES = 8
BPG = 32                 # graphs per block
NBLK = G // BPG          # 128 blocks total
BPC = NBLK // NCORES     # 16 blocks per core
SENT = 999.0             # rel-batch sentinel for padded nodes

LAST_EXEC_TIME_NS = None
LAST_TRACE = None
_NC_CACHE = {}


def _bview(ap, tail):
    p = ap.ap[0]
    return bass.AP(ap.tensor, ap.offset, [[p[0], p[1]]] + [list(d) for d in tail])


def _build_body(ctx, tc, aps, BS):
    nc = tc.nc
    f32 = mybir.dt.float32
    bf16 = mybir.dt.bfloat16
    xn_d, rm_d, w1t_d, w2c_d, b1_d, b2_d, id_d, e_d, out_d = aps
    SQ = BS              # superquads per core (2048 nodes each)
    assert SQ % 2 == 0

    PSUM = bass.MemorySpace.PSUM
    cpool = ctx.enter_context(tc.tile_pool(name="cpool", bufs=1))
    xn_pool = ctx.enter_context(tc.tile_pool(name="xn_pool", bufs=9))
    xtp_pool = ctx.enter_context(tc.tile_pool(name="xtp_pool", bufs=2,
                                              space=PSUM))
    xts_pool = ctx.enter_context(tc.tile_pool(name="xts_pool", bufs=4))
    hps_pool = ctx.enter_context(tc.tile_pool(name="hps_pool", bufs=2,
                                              space=PSUM))
    ht_pool = ctx.enter_context(tc.tile_pool(name="ht_pool", bufs=3))

    e_pool = ctx.enter_context(tc.tile_pool(name="e_pool", bufs=2))
    rm_pool = ctx.enter_context(tc.tile_pool(name="rm_pool", bufs=5))
    r_pool = ctx.enter_context(tc.tile_pool(name="r_pool", bufs=3))
    wps_pool = ctx.enter_context(tc.tile_pool(name="wps_pool", bufs=1,
                                              space=PSUM))
    sps_pool = ctx.enter_context(tc.tile_pool(name="sps_pool", bufs=1,
                                              space=PSUM))
    o_pool = ctx.enter_context(tc.tile_pool(name="o_pool", bufs=2))

    xn_tiles, xts_tiles, ht_tiles = {}, {}, {}
    rm_tiles, e_tiles, r_tiles, sps_tiles = {}, {}, {}, {}
    for j0 in (0, 1):
        xn0 = xn_pool.tile([128, 2048], bf16, name="xn")
        nc.sync.dma_start(xn0[:], xn_d[j0])
        xn_tiles[j0] = xn0
    w1t = cpool.tile([128, 128], bf16)
    nc.sync.dma_start(w1t[:], w1t_d[:])
    w2c = cpool.tile([128, 1], bf16)
    nc.sync.dma_start(w2c[:], w2c_d[:])
    b1 = cpool.tile([128, 1], f32)
    nc.sync.dma_start(b1[:], b1_d[:])
    b2 = cpool.tile([128, 1], f32)
    nc.sync.dma_start(b2[:], b2_d[:])
    ident = cpool.tile([128, 128], bf16)
    nc.sync.dma_start(ident[:], id_d[:])

    win = [None]
    ob = [None]

    # per-engine stage skews (iteration offsets)
    T_TRANS, T_H, T_S, T_MULT, T_POOL = 2, 3, 4, 5, 7

    for it in range(SQ + T_POOL + 1):
        # ---- DMA: load superquad `it` ----
        if it < SQ:
            if it >= 2:
                xn = xn_pool.tile([128, 2048], bf16, name="xn")
                nc.sync.dma_start(xn[:], xn_d[it])
                xn_tiles[it] = xn
            if it % 2 == 0:
                rm = rm_pool.tile([128, 1024], bf16)
                nc.sync.dma_start(rm[:], rm_d[it // 2])
                rm_tiles[it // 2] = rm

        # ---- PE transposes + DVE copies + GPS eq ----
        j = it - T_TRANS
        if 0 <= j < SQ:
            xnj = xn_tiles[j]
            ta = xtp_pool.tile([128, 1024], bf16, name="tp")
            tb = xtp_pool.tile([128, 1024], bf16, name="tp")
            for i in range(8):
                nc.tensor.transpose(ta[:, 128 * i:128 * i + 128],
                                    xnj[:, 128 * i:128 * i + 128], ident[:])
            for i in range(8):
                o = 128 * (8 + i)
                nc.tensor.transpose(tb[:, 128 * i:128 * i + 128],
                                    xnj[:, o:o + 128], ident[:])
            xts = xts_pool.tile([128, 2048], bf16)
            nc.vector.tensor_copy(xts[:, 0:1024], ta[:])
            nc.vector.tensor_copy(xts[:, 1024:2048], tb[:])
            xts_tiles[j] = xts

        # ---- PE h-matmul + ACT tanh ----
        j = it - T_H
        if 0 <= j < SQ:
            xts = xts_tiles[j]
            ht = ht_pool.tile([128, 2048], bf16)
            for half in range(2):
                hps = hps_pool.tile([128, 1024], f32)
                for qq in range(2):
                    off = 1024 * half + 512 * qq
                    nc.tensor.matmul(hps[:, 512 * qq:512 * qq + 512], w1t[:],
                                     xts[:, off:off + 512],
                                     start=True, stop=True)
                nc.scalar.activation(ht[:, 1024 * half:1024 * half + 1024],
                                     hps[:],
                                     mybir.ActivationFunctionType.Tanh,
                                     bias=b1[:])
            ht_tiles[j] = ht
            del xts_tiles[j]

        # ---- PE s-matmuls (+ ACT exp, DMA e-write per pair) ----
        j = it - T_S
        if 0 <= j < SQ:
            k = j // 2
            if j % 2 == 0:
                sps_tiles[k] = sps_pool.tile([128, 32], f32, name="sps")
            sps = sps_tiles[k]
            ht = ht_tiles[j]
            for i in range(16):
                col = 16 * (j % 2) + i
                nc.tensor.matmul(sps[:, col:col + 1],
                                 ht[:, 128 * i:128 * i + 128], w2c[:],
                                 start=True, stop=True)
            del ht_tiles[j]
            if j % 2 == 1:
                e2 = e_pool.tile([128, 32], f32)
                nc.scalar.activation(e2[:], sps[:],
                                     mybir.ActivationFunctionType.Exp,
                                     bias=b2[:])
                nc.sync.dma_start(e_d[k], e2[:])
                e_tiles[k] = e2
                del sps_tiles[k]

        # ---- DVE mult: r16 = rmask * e (pair k) ----
        j = it - T_MULT
        if 0 <= j < SQ and j % 2 == 1:
            k = j // 2
            r16 = r_pool.tile([128, 1024], bf16)
            nc.vector.tensor_tensor(
                _bview(r16[:], [[32, 32], [1, 32]]),
                _bview(rm_tiles[k][:], [[32, 32], [1, 32]]),
                _bview(e_tiles[k][:], [[1, 32], [0, 32]]),
                mybir.AluOpType.mult)
            r_tiles[k] = r16
            del rm_tiles[k]
            del e_tiles[k]

        # ---- PE pooled matmuls (+ DVE win->ob copy, DMA out) ----
        jp = it - T_POOL
        if 0 <= jp < SQ and jp % 2 == 0:
            k = jp // 2
            r16 = r_tiles[k]
            for j in (jp, jp + 1):
                xnj = xn_tiles[j]
                for i in range(16):
                    t = 16 * j + i
                    b = t // BS
                    first = (t == b * BS)
                    last = (t == (b + 1) * BS - 1)
                    if first:
                        win[0] = wps_pool.tile([128, 32], f32, name="win")
                    rcol = 32 * (t - 32 * k)
                    nc.tensor.matmul(win[0][:],
                                     xnj[:, 128 * i:128 * i + 128],
                                     r16[:, rcol:rcol + 32],
                                     start=first, stop=last)
                    if last:
                        if b % 2 == 0:
                            ob[0] = o_pool.tile([128, 64], f32, name="ob")
                        sl = 32 * (b % 2)
                        nc.vector.tensor_copy(ob[0][:, sl:sl + 32], win[0][:])
                        if b % 2 == 1:
                            nc.sync.dma_start(out_d[b // 2], ob[0][:])
                del xn_tiles[j]
            del r_tiles[k]


def _build_nc(BS):
    key = BS
    if key in _NC_CACHE:
        return _NC_CACHE[key]
    dt = mybir.dt
    nc = bacc.Bacc("TRN2", target_bir_lowering=False, debug=False,
                   enable_asserts=False, num_devices=NCORES)
    xn_d = nc.dram_tensor("x_n", [BS, 128, 2048], dt.bfloat16,
                          kind="ExternalInput").ap()
    rm_d = nc.dram_tensor("rmask", [BS // 2, 128, 1024], dt.bfloat16,
                          kind="ExternalInput").ap()
    w1t_d = nc.dram_tensor("w1t", [128, 128], dt.bfloat16,
                           kind="ExternalInput").ap()
    w2c_d = nc.dram_tensor("w2c", [128, 1], dt.bfloat16,
                           kind="ExternalInput").ap()
    b1_d = nc.dram_tensor("b1", [128, 1], dt.float32,
                          kind="ExternalInput").ap()
    b2_d = nc.dram_tensor("b2", [128, 1], dt.float32,
                          kind="ExternalInput").ap()
    id_d = nc.dram_tensor("ident", [128, 128], dt.bfloat16,
                          kind="ExternalInput").ap()
    e_d = nc.dram_tensor("e_out", [BS // 2, 128, 32], dt.float32,
                         kind="ExternalOutput").ap()
    out_d = nc.dram_tensor("outT", [BPC // 2, 128, 64], dt.float32,
                           kind="ExternalOutput").ap()
    aps = (xn_d, rm_d, w1t_d, w2c_d, b1_d, b2_d, id_d, e_d, out_d)

    import contextlib
    with tile.TileContext(nc) as tc:
        with contextlib.ExitStack() as ctx:
            _build_body(ctx, tc, aps, BS)
    nc.compile()
    _NC_CACHE[key] = nc
    return nc


def kernel(x, proj_w, proj_b, score_w, score_b, batch, num_graphs):
    global LAST_EXEC_TIME_NS, LAST_TRACE
    assert x.shape == (N, C)
    assert int(num_graphs) == G

    batch = np.asarray(batch).astype(np.int64)
    counts = np.bincount(batch, minlength=G)
    blk_cnt = counts.reshape(NBLK, BPG).sum(1)                    # [128]
    BS = max(1, int(-(-int(blk_cnt.max()) // 128)))               # subtiles/block
    BS += BS % 2                                                  # even BS
    ST = 16 * BS

    starts = np.concatenate([[0], np.cumsum(counts)])
    blk_start = starts[0:G:BPG][:NBLK]

    pos = np.arange(BS * 128)
    valid = pos[None, :] < blk_cnt[:, None]                       # [128, BS*128]
    idx = blk_start[:, None] + pos[None, :]                       # [128, BS*128]

    x_bf = np.zeros((NBLK, BS * 128, C), dtype=BF16)
    x_bf[valid] = np.asarray(x, dtype=np.float32)[idx[valid]].astype(BF16)
    rel = np.full((NBLK, BS * 128), SENT, dtype=np.float32)
    blk_of = np.nonzero(valid)[0]
    rel[valid] = (batch[idx[valid]] - BPG * blk_of).astype(np.float32)

    xc = x_bf.reshape(NCORES, BS, 2048, C)                         # [8,BS,2048,128]
    xn_r = np.ascontiguousarray(
        xc.reshape(NCORES, BS, 16, 128, C).transpose(0, 1, 3, 2, 4)
        .reshape(NCORES, BS, 128, 2048))
    rel_r = rel.reshape(NCORES, ST, 128)                           # [8, ST, 128]
    onehot = (rel_r[:, :, :, None] ==
              np.arange(BPG, dtype=np.float32)[None, None, None, :])
    rmask = np.ascontiguousarray(
        onehot.reshape(NCORES, BS // 2, 32, 128, 32)
        .transpose(0, 1, 3, 2, 4)
        .reshape(NCORES, BS // 2, 128, 1024).astype(BF16))

    w1t = np.ascontiguousarray(np.asarray(proj_w, np.float32).T).astype(BF16)
    w2c = np.ascontiguousarray(
        np.asarray(score_w, np.float32)[0][:, None]).astype(BF16)
    b1c = np.ascontiguousarray(np.asarray(proj_b, np.float32)[:, None])
    b2c = np.full((128, 1), np.asarray(score_b, np.float32)[0],
                  dtype=np.float32)
    ident = np.eye(128, dtype=BF16)

    nc = _build_nc(BS)

    in_maps = [{
        "x_n": xn_r[d], "rmask": rmask[d],
        "w1t": w1t, "w2c": w2c, "b1": b1c, "b2": b2c,
        "ident": ident,
    } for d in range(NCORES)]

    trace = bool(os.environ.get("NAP_TRACE"))
    try:
        res = run_bass_kernel_spmd(nc, in_maps, list(range(NCORES)),
                                   trace=trace)
    except Exception:
        if not trace:
            raise
        import traceback
        traceback.print_exc()
        print("trace run failed; retrying without trace", file=sys.stderr)
        res = run_bass_kernel_spmd(nc, in_maps, list(range(NCORES)),
                                   trace=False)
    LAST_EXEC_TIME_NS = res.exec_time_ns
    LAST_TRACE = res.instructions_and_trace

    poolT = np.stack([np.asarray(res.results[d]["outT"], dtype=np.float32)
                      for d in range(NCORES)])                     # [8,8,128,64]
    pooled = (poolT.reshape(NCORES, BPC // 2, 128, 2, 32)
              .transpose(0, 1, 3, 4, 2).reshape(G, C))             # [4096,128]
    e_all = np.stack([np.asarray(res.results[d]["e_out"], dtype=np.float32)
                      for d in range(NCORES)])                     # [8,BS/2,128,32]
    e_flat = e_all.transpose(0, 1, 3, 2).reshape(-1)               # padded order
    eb = e_flat.astype(BF16).astype(np.float32)
    gid = (rel + (BPG * np.arange(NBLK, dtype=np.float32))[:, None]).reshape(-1)
    vm = valid.reshape(-1)
    den = np.bincount(gid[vm].astype(np.int64), weights=eb[vm], minlength=G)
    out = pooled / np.maximum(den, 1e-30)[:, None].astype(np.float32)
    return out.astype(np.float32)


# revision 17
# speedup vs baseline: 1.0399x; 1.0399x over previous
import os
import sys

import numpy as np

sys.path.insert(0, "/opt/trn_rl_repo")

import ml_dtypes  # noqa: E402

import concourse.bass as bass  # noqa: E402
import concourse.tile as tile  # noqa: E402
from concourse import bacc, mybir  # noqa: E402
from concourse.bass_utils import run_bass_kernel_spmd  # noqa: E402

BF16 = ml_dtypes.bfloat16
N, C, G = 500000, 128, 4096
NCORES = 8
BPG = 32                 # graphs per block
NBLK = G // BPG          # 128 blocks total
BPC = NBLK // NCORES     # 16 blocks per core
SENT = 999.0             # rel-batch sentinel for padded nodes

LAST_EXEC_TIME_NS = None
LAST_TRACE = None
_NC_CACHE = {}


def _bview(ap, tail):
    p = ap.ap[0]
    return bass.AP(ap.tensor, ap.offset, [[p[0], p[1]]] + [list(d) for d in tail])


def _build_body(ctx, tc, aps, BS):
    nc = tc.nc
    f32 = mybir.dt.float32
    bf16 = mybir.dt.bfloat16
    xn_d, rm_d, w1t_d, w2c_d, b1_d, b2_d, id_d, e_d, out_d = aps
    SQ = BS              # superquads per core (2048 nodes each)
    assert SQ % 2 == 0

    PSUM = bass.MemorySpace.PSUM
    cpool = ctx.enter_context(tc.tile_pool(name="cpool", bufs=1))
    xn_pool = ctx.enter_context(tc.tile_pool(name="xn_pool", bufs=9))
    xtp_pool = ctx.enter_context(tc.tile_pool(name="xtp_pool", bufs=2,
                                              space=PSUM))
    xts_pool = ctx.enter_context(tc.tile_pool(name="xts_pool", bufs=4))
    hps_pool = ctx.enter_context(tc.tile_pool(name="hps_pool", bufs=2,
                                              space=PSUM))
    ht_pool = ctx.enter_context(tc.tile_pool(name="ht_pool", bufs=3))

    e_pool = ctx.enter_context(tc.tile_pool(name="e_pool", bufs=2))
    rm_pool = ctx.enter_context(tc.tile_pool(name="rm_pool", bufs=5))
    r_pool = ctx.enter_context(tc.tile_pool(name="r_pool", bufs=3))
    wps_pool = ctx.enter_context(tc.tile_pool(name="wps_pool", bufs=1,
                                              space=PSUM))
    sps_pool = ctx.enter_context(tc.tile_pool(name="sps_pool", bufs=1,
                                              space=PSUM))
    o_pool = ctx.enter_context(tc.tile_pool(name="o_pool", bufs=2))

    xn_tiles, xts_tiles, ht_tiles = {}, {}, {}
    rm_tiles, e_tiles, r_tiles, sps_tiles = {}, {}, {}, {}
    for j0 in (0, 1):
        xn0 = xn_pool.tile([128, 2048], bf16, name="xn")
        nc.sync.dma_start(xn0[:], xn_d[j0])
        xn_tiles[j0] = xn0
    w1t = cpool.tile([128, 128], bf16)
    nc.sync.dma_start(w1t[:], w1t_d[:])
    w2c = cpool.tile([128, 1], bf16)
    nc.sync.dma_start(w2c[:], w2c_d[:])
    b1 = cpool.tile([128, 1], f32)
    nc.sync.dma_start(b1[:], b1_d[:])
    b2 = cpool.tile([128, 1], f32)
    nc.sync.dma_start(b2[:], b2_d[:])
    ident = cpool.tile([128, 128], bf16)
    nc.sync.dma_start(ident[:], id_d[:])

    win = [None]
    ob = [None]

    # per-engine stage skews (iteration offsets)
    T_TRANS, T_H, T_S, T_MULT, T_POOL = 2, 3, 4, 5, 7

    for it in range(SQ + T_POOL + 1):
        # ---- DMA: load superquad `it` ----
        if it < SQ:
            if it >= 2:
                xn = xn_pool.tile([128, 2048], bf16, name="xn")
                nc.sync.dma_start(xn[:], xn_d[it])
                xn_tiles[it] = xn
            if it % 2 == 0:
                rm = rm_pool.tile([128, 1024], bf16)
                nc.sync.dma_start(rm[:], rm_d[it // 2])
                rm_tiles[it // 2] = rm

        # ---- PE transposes + DVE copies + GPS eq ----
        j = it - T_TRANS
        if 0 <= j < SQ:
            xnj = xn_tiles[j]
            ta = xtp_pool.tile([128, 1024], bf16, name="tp")
            tb = xtp_pool.tile([128, 1024], bf16, name="tp")
            for i in range(8):
                nc.tensor.transpose(ta[:, 128 * i:128 * i + 128],
                                    xnj[:, 128 * i:128 * i + 128], ident[:])
            for i in range(8):
                o = 128 * (8 + i)
                nc.tensor.transpose(tb[:, 128 * i:128 * i + 128],
                                    xnj[:, o:o + 128], ident[:])
            xts = xts_pool.tile([128, 2048], bf16)
            nc.vector.tensor_copy(xts[:, 0:1024], ta[:])
            nc.vector.tensor_copy(xts[:, 1024:2048], tb[:])
            xts_tiles[j] = xts

        # ---- PE h-matmul + ACT tanh ----
        jh = it - T_H
        if 0 <= jh < SQ:
            xts = xts_tiles[jh]
            ht = ht_pool.tile([128, 2048], bf16)
            for half in range(2):
                hps = hps_pool.tile([128, 1024], f32)
                for qq in range(2):
                    off = 1024 * half + 512 * qq
                    nc.tensor.matmul(hps[:, 512 * qq:512 * qq + 512], w1t[:],
                                     xts[:, off:off + 512],
                                     start=True, stop=True)
                nc.scalar.activation(ht[:, 1024 * half:1024 * half + 1024],
                                     hps[:],
                                     mybir.ActivationFunctionType.Tanh,
                                     bias=b1[:])
            ht_tiles[jh] = ht
            del xts_tiles[jh]

        # ---- PE s-matmuls (+ ACT exp, DMA e-write per pair) ----
        js = it - T_S
        if 0 <= js < SQ:
            ks = js // 2
            if js % 2 == 0:
                sps_tiles[ks] = sps_pool.tile([128, 32], f32, name="sps")
            sps = sps_tiles[ks]
            hts = ht_tiles[js]
            for i in range(16):
                col = 16 * (js % 2) + i
                nc.tensor.matmul(sps[:, col:col + 1],
                                 hts[:, 128 * i:128 * i + 128], w2c[:],
                                 start=True, stop=True)
            del ht_tiles[js]
            if js % 2 == 1:
                e2 = e_pool.tile([128, 32], f32)
                nc.scalar.activation(e2[:], sps[:],
                                     mybir.ActivationFunctionType.Exp,
                                     bias=b2[:])
                nc.sync.dma_start(e_d[ks], e2[:])
                e_tiles[ks] = e2
                del sps_tiles[ks]

        # ---- DVE mult: r16 = rmask * e (pair k) ----
        j = it - T_MULT
        if 0 <= j < SQ and j % 2 == 1:
            k = j // 2
            r16 = r_pool.tile([128, 1024], bf16)
            nc.vector.tensor_tensor(
                _bview(r16[:], [[32, 32], [1, 32]]),
                _bview(rm_tiles[k][:], [[32, 32], [1, 32]]),
                _bview(e_tiles[k][:], [[1, 32], [0, 32]]),
                mybir.AluOpType.mult)
            r_tiles[k] = r16
            del rm_tiles[k]
            del e_tiles[k]

        # ---- PE pooled matmuls (+ DVE win->ob copy, DMA out) ----
        jp = it - T_POOL
        if 0 <= jp < SQ and jp % 2 == 0:
            k = jp // 2
            r16 = r_tiles[k]
            for j in (jp, jp + 1):
                xnj = xn_tiles[j]
                for i in range(16):
                    t = 16 * j + i
                    b = t // BS
                    first = (t == b * BS)
                    last = (t == (b + 1) * BS - 1)
                    if first:
                        win[0] = wps_pool.tile([128, 32], f32, name="win")
                    rcol = 32 * (t - 32 * k)
                    nc.tensor.matmul(win[0][:],
                                     xnj[:, 128 * i:128 * i + 128],
                                     r16[:, rcol:rcol + 32],
                                     start=first, stop=last)
                    if last:
                        if b % 2 == 0:
                            ob[0] = o_pool.tile([128, 64], f32, name="ob")
                        sl = 32 * (b % 2)
                        nc.vector.tensor_copy(ob[0][:, sl:sl + 32], win[0][:])
                        if b % 2 == 1:
                            nc.sync.dma_start(out_d[b // 2], ob[0][:])
                del xn_tiles[j]
            del r_tiles[k]


def _build_nc(BS):
    key = BS
    if key in _NC_CACHE:
        return _NC_CACHE[key]
    dt = mybir.dt
    nc = bacc.Bacc("TRN2", target_bir_lowering=False, debug=False,
                   enable_asserts=False, num_devices=NCORES)
    xn_d = nc.dram_tensor("x_n", [BS, 128, 2048], dt.bfloat16,
                          kind="ExternalInput").ap()
    rm_d = nc.dram_tensor("rmask", [BS // 2, 128, 1024], dt.bfloat16,
                          kind="ExternalInput").ap()
    w1t_d = nc.dram_tensor("w1t", [128, 128], dt.bfloat16,
                           kind="ExternalInput").ap()
    w2c_d = nc.dram_tensor("w2c", [128, 1], dt.bfloat16,
                           kind="ExternalInput").ap()
    b1_d = nc.dram_tensor("b1", [128, 1], dt.float32,
                          kind="ExternalInput").ap()
    b2_d = nc.dram_tensor("b2", [128, 1], dt.float32,
                          kind="ExternalInput").ap()
    id_d = nc.dram_tensor("ident", [128, 128], dt.bfloat16,
                          kind="ExternalInput").ap()
    e_d = nc.dram_tensor("e_out", [BS // 2, 128, 32], dt.float32,
                         kind="ExternalOutput").ap()
    out_d = nc.dram_tensor("outT", [BPC // 2, 128, 64], dt.float32,
                           kind="ExternalOutput").ap()
    aps = (xn_d, rm_d, w1t_d, w2c_d, b1_d, b2_d, id_d, e_d, out_d)

    import contextlib
    with tile.TileContext(nc) as tc:
        with contextlib.ExitStack() as ctx:
            _build_body(ctx, tc, aps, BS)
    nc.compile()
    _NC_CACHE[key] = nc
    return nc


def kernel(x, proj_w, proj_b, score_w, score_b, batch, num_graphs):
    global LAST_EXEC_TIME_NS, LAST_TRACE
    assert x.shape == (N, C)
    assert int(num_graphs) == G

    batch = np.asarray(batch).astype(np.int64)
    counts = np.bincount(batch, minlength=G)
    blk_cnt = counts.reshape(NBLK, BPG).sum(1)                    # [128]
    BS = max(1, int(-(-int(blk_cnt.max()) // 128)))               # subtiles/block
    BS += BS % 2                                                  # even BS
    ST = 16 * BS

    starts = np.concatenate([[0], np.cumsum(counts)])
    blk_start = starts[0:G:BPG][:NBLK]

    pos = np.arange(BS * 128)
    valid = pos[None, :] < blk_cnt[:, None]                       # [128, BS*128]
    idx = blk_start[:, None] + pos[None, :]                       # [128, BS*128]

    x_bf = np.zeros((NBLK, BS * 128, C), dtype=BF16)
    x_bf[valid] = np.asarray(x, dtype=np.float32)[idx[valid]].astype(BF16)
    rel = np.full((NBLK, BS * 128), SENT, dtype=np.float32)
    blk_of = np.nonzero(valid)[0]
    rel[valid] = (batch[idx[valid]] - BPG * blk_of).astype(np.float32)

    xc = x_bf.reshape(NCORES, BS, 2048, C)                         # [8,BS,2048,128]
    xn_r = np.ascontiguousarray(
        xc.reshape(NCORES, BS, 16, 128, C).transpose(0, 1, 3, 2, 4)
        .reshape(NCORES, BS, 128, 2048))
    rel_r = rel.reshape(NCORES, ST, 128)                           # [8, ST, 128]
    onehot = (rel_r[:, :, :, None] ==
              np.arange(BPG, dtype=np.float32)[None, None, None, :])
    rmask = np.ascontiguousarray(
        onehot.reshape(NCORES, BS // 2, 32, 128, 32)
        .transpose(0, 1, 3, 2, 4)
        .reshape(NCORES, BS // 2, 128, 1024).astype(BF16))

    w1t = np.ascontiguousarray(np.asarray(proj_w, np.float32).T).astype(BF16)
    w2c = np.ascontiguousarray(
        np.asarray(score_w, np.float32)[0][:, None]).astype(BF16)
    b1c = np.ascontiguousarray(np.asarray(proj_b, np.float32)[:, None])
    b2c = np.full((128, 1), np.asarray(score_b, np.float32)[0],
                  dtype=np.float32)
    ident = np.eye(128, dtype=BF16)

    nc = _build_nc(BS)

    in_maps = [{
        "x_n": xn_r[d], "rmask": rmask[d],
        "w1t": w1t, "w2c": w2c, "b1": b1c, "b2": b2c,
        "ident": ident,
    } for d in range(NCORES)]

    trace = bool(os.environ.get("NAP_TRACE"))
    try:
        res = run_bass_kernel_spmd(nc, in_maps, list(range(NCORES)),
                                   trace=trace)
    except Exception:
        if not trace:
            raise
        import traceback
        traceback.print_exc()
        print("trace run failed; retrying without trace", file=sys.stderr)
        res = run_bass_kernel_spmd(nc, in_maps, list(range(NCORES)),
                                   trace=False)
    LAST_EXEC_TIME_NS = res.exec_time_ns
    LAST_TRACE = res.instructions_and_trace

    poolT = np.stack([np.asarray(res.results[d]["outT"], dtype=np.float32)
                      for d in range(NCORES)])                     # [8,8,128,64]
    pooled = (poolT.reshape(NCORES, BPC // 2, 128, 2, 32)
              .transpose(0, 1, 3, 4, 2).reshape(G, C))             # [4096,128]
    e_all = np.stack([np.asarray(res.results[d]["e_out"], dtype=np.float32)
                      for d in range(NCORES)])                     # [8,BS/2,128,32]
    e_flat = e_all.transpose(0, 1, 3, 2).reshape(-1)               # padded order
    eb = e_flat.astype(BF16).astype(np.float32)
    gid = (rel + (BPG * np.arange(NBLK, dtype=np.float32))[:, None]).reshape(-1)
    vm = valid.reshape(-1)
    den = np.bincount(gid[vm].astype(np.int64), weights=eb[vm], minlength=G)
    out = pooled / np.maximum(den, 1e-30)[:, None].astype(np.float32)
    return out.astype(np.float32)
